# revision 17
# baseline (speedup 1.0000x reference)
"""Trainium2 Bass kernel for nn_DifferentiableStarPlanner.

Algorithm notes (validated bitwise vs the reference in numpy):

  * The reference's open/close/pool computations never feed the returned
    tensor: the output is exactly NUM_SWEEPS Jacobi sweeps of a 9-channel
    min-plus stencil  g <- min(g, min_c(shift_c(g) + cmap_c))  with
    g0 = 1e7 everywhere except the start cell.
  * Information propagates one cell per sweep from the start cell, so only
    the bounding box of the start support, inflated by NUM_SWEEPS and clipped
    to the grid, can ever change from 1e7.  For the shipped problem that is
    rows/cols 0..112 (a 113x113 corner of the 512x512 grid).
  * Edge-replicate padding can be replaced by +inf (1e7) guard cells: at a
    true grid edge every clamped channel is either bitwise-identical to
    another channel or provably >= it (fp32 ops used are monotone), so
    dropping them never changes the min.  Same for the center channel
    (cmap >= 0).  This leaves 8 shift channels and static 1e7 guards.
  * Per sweep only cells within t steps of the start can change, so the
    active window grows by one cell per sweep.

Device mapping (one NeuronCore; all 8 cores run identical replicas).
The state alternates orientation every sweep, which makes every neighbor
shift expressible as ONE TensorEngine transpose-mode matmul (pure routing,
bit-exact) directly from the state:

  * odd sweeps:  state g_rm [rows, cols]; channel (dy,dx) is
       psumT[dy,dx] = T( g_rm[:, dx-window] ) with row-cyclic permutation
    P_dy as the moving operand, accumulated onto a cmap preload: the column
    shift is a free-dim slice of the stationary operand, the row shift the
    permutation (its cycle wrap lands on a 1e7 junk lane = domain guard).
    Output (and hence the next state s_T) is column-major.
  * even sweeps: symmetric, from s_T back to row-major.
  * A 9th "pure copy" channel (identity permutation, no cmap) injects g
    itself into the 9-way DVE min-reduce, whose output IS the next state:
    per sweep the DVE runs exactly one instruction.
  * cmap for sweep t+1 is preloaded into the other PSUM bank set during
    sweep t (8 transpose matmuls, overlapped with the reduce).
"""
import sys
import os
import numpy as np

for _p in ("/opt/trn_rl_repo", "/root/.axon_site/_ro/trn_rl_repo"):
    if os.path.isdir(_p) and _p not in sys.path:
        sys.path.insert(0, _p)

import concourse.bass as bass
import concourse.bacc as bacc
import concourse.mybir as mybir
from concourse import tile
from concourse.bass_utils import run_bass_kernel_spmd

F32 = mybir.dt.float32
ALU = mybir.AluOpType
AXL = mybir.AxisListType
ACTF = mybir.ActivationFunctionType

INF = np.float32(1.0e7)
OC = float(np.float32(10000.0))
EPS_F = np.float32(1e-12)
NUM_SWEEPS = 80
N_CORES = 8

# channels: (dy, dx), center excluded
CHANNELS = [(dy, dx) for dy in (-1, 0, 1) for dx in (-1, 0, 1) if not (dy == 0 and dx == 0)]
SS = 116  # psum region stride within a bank


def build_program(Dr, Dc, seed_rlo, seed_rhi, seed_clo, seed_chi, r0, c0,
                  H, W, num_sweeps):
    """Domain = grid rows r0..r0+Dr-1, cols c0..c0+Dc-1; seed_* in domain coords."""
    Sr, Sc = Dr + 2, Dc + 2
    KR, KC = Dr + 1, Dc + 1      # state partition counts incl junk/guard lane
    assert KR <= 115 and KC <= 115 and Sc <= 128 and 3 * SS <= 1536

    nc = bacc.Bacc("TRN2", target_bir_lowering=False, debug=False)

    # ---- DRAM I/O (inputs packed: single DMA) ----
    seg = [("obsT", Sr), ("obsTm", Sr), ("obsTp", Sr), ("xcT", Sr), ("xcTm", Sr),
           ("xcTp", Sr), ("ycT", Sr), ("startm", Dc), ("ident", Sc), ("sig", Sc),
           ("crm", KR), ("crp", KR), ("ccm", KC), ("ccp", KC)]
    offs, TOT = {}, 0
    for nm, wd in seg:
        offs[nm] = TOT
        TOT += wd
    d_pack = nc.dram_tensor("packed", [Sc, TOT], F32, kind="ExternalInput")
    d_out = nc.dram_tensor("out", [H, W], F32, kind="ExternalOutput")

    with tile.TileContext(nc) as tc:
        from contextlib import ExitStack
        with ExitStack() as ctx:
            sb = ctx.enter_context(tc.tile_pool(name="sb", bufs=1))
            ps = ctx.enter_context(tc.tile_pool(name="ps", bufs=1, space="PSUM"))

            # ---- SBUF tiles ----
            t_all = sb.tile([Sc, TOT], F32)
            t_in = {nm: t_all[:, offs[nm]:offs[nm] + Sr] for nm in
                    ("obsT", "obsTm", "obsTp", "xcT", "xcTm", "xcTp", "ycT")}
            t_start = t_all[0:Dr, offs["startm"]:offs["startm"] + Dc]
            # DVE-owned copies of the constant matrices
            identC = sb.tile([Sc, Sc], F32)
            sigC = sb.tile([Sc, Sc], F32)
            crmC = sb.tile([KR, KR], F32)
            crpC = sb.tile([KR, KR], F32)
            ccmC = sb.tile([KC, KC], F32)
            ccpC = sb.tile([KC, KC], F32)
            g_rm = sb.tile([KR, Dc + 3], F32)   # rows+junk | colguard,cols,2 guards
            s_T = sb.tile([KC, Dr + 3], F32)    # cols+junk | rowguard,rows,2 guards
            bg = sb.tile([128, W], F32)
            bias_eps = sb.tile([Sc, 1], F32)
            sq = {nm: sb.tile([Sc, Dr], F32, name=f"sq_{nm}") for nm in ("L", "R", "U", "D")}
            t_tmp = sb.tile([Sc, Dr], F32)
            t_A = {ch: sb.tile([Sc, Dr], F32, name=f"A_{ch[0]+1}{ch[1]+1}") for ch in CHANNELS}
            t_mx = {ch: sb.tile([Sc, Dr], F32, name=f"mx_{ch[0]+1}{ch[1]+1}") for ch in CHANNELS}
            # transposed cmap (+1e7 junk-row slot); partitions = cols -1..Dc
            t_cmapT = {ch: sb.tile([Sc, KR], F32, name=f"cmapT_{ch[0]+1}{ch[1]+1}")
                       for ch in CHANNELS}
            # row-major cmap (+1e7 junk-col lane); partitions = rows 0..Dr
            t_cmapR = {ch: sb.tile([KR, KC], F32, name=f"cmapR_{ch[0]+1}{ch[1]+1}")
                       for ch in CHANNELS}

            # ---- PSUM: two bank sets of 3 banks (3 regions each) ----
            psum_sets = [ps.tile([128, 1536], F32, name="psumA"),
                         ps.tile([128, 1536], F32, name="psumB")]
            psD = ps.tile([128, 512], F32, name="psD")
            t_warm = sb.tile([128, 512], mybir.dt.bfloat16)

            # ---- load inputs (single DMA) + const copies ----
            nc.sync.dma_start(t_all[:], d_pack.ap())
            v = nc.vector
            v.tensor_copy(identC[:], t_all[:, offs["ident"]:offs["ident"] + Sc])
            v.tensor_copy(sigC[:], t_all[:, offs["sig"]:offs["sig"] + Sc])
            v.tensor_copy(crmC[:], t_all[0:KR, offs["crm"]:offs["crm"] + KR])
            v.tensor_copy(crpC[:], t_all[0:KR, offs["crp"]:offs["crp"] + KR])
            v.tensor_copy(ccmC[:], t_all[0:KC, offs["ccm"]:offs["ccm"] + KC])
            v.tensor_copy(ccpC[:], t_all[0:KC, offs["ccp"]:offs["ccp"] + KC])

            # ---- init ----
            v.memset(t_warm[:], 1.0)
            v.memset(bg[:], INF)
            v.memset(g_rm[:], INF)
            v.memset(s_T[:], INF)
            v.memset(bias_eps[:], EPS_F)

            # ---- background writes (1e7 outside the domain) ----
            out_ap = d_out.ap()
            bg_rows = []
            if r0 > 0:
                bg_rows.append((0, r0))
            if r0 + Dr < H:
                bg_rows.append((r0 + Dr, H))
            for lo_, hi_ in bg_rows:
                r = lo_
                while r < hi_:
                    n = min(128, hi_ - r)
                    nc.sync.dma_start(out_ap[r:r + n, :], bg[0:n, :])
                    r += n
            if c0 > 0:
                nc.sync.dma_start(out_ap[r0:r0 + Dr, 0:c0], bg[0:Dr, 0:c0])
            if c0 + Dc < W:
                nc.sync.dma_start(out_ap[r0:r0 + Dr, c0 + Dc:W],
                                  bg[0:Dr, 0:W - c0 - Dc])

            # ---- cmap channels, computed in transposed orientation ----
            rows = slice(1, 1 + Dr)
            v.tensor_sub(t_tmp[:], t_in["xcT"][:, rows], t_in["xcTm"][:, rows])
            v.tensor_mul(sq["L"][:], t_tmp[:], t_tmp[:])
            v.tensor_sub(t_tmp[:], t_in["xcT"][:, rows], t_in["xcTp"][:, rows])
            v.tensor_mul(sq["R"][:], t_tmp[:], t_tmp[:])
            v.tensor_sub(t_tmp[:], t_in["ycT"][:, rows], t_in["ycT"][:, 2:2 + Dr])
            v.tensor_mul(sq["U"][:], t_tmp[:], t_tmp[:])
            v.tensor_sub(t_tmp[:], t_in["ycT"][:, rows], t_in["ycT"][:, 0:Dr])
            v.tensor_mul(sq["D"][:], t_tmp[:], t_tmp[:])

            geo = {(-1, -1): ("L", "U"), (0, -1): ("L",), (1, -1): ("L", "D"),
                   (-1, 0): ("U",), (1, 0): ("D",),
                   (-1, 1): ("R", "U"), (0, 1): ("R",), (1, 1): ("R", "D")}
            obsnb = {(-1, -1): (-1, -1), (0, -1): (-1, 0), (1, -1): (1, -1),
                     (-1, 0): (-1, 0), (1, 0): (1, 0),
                     (-1, 1): (-1, 1), (0, 1): (0, 1), (1, 1): (1, 1)}
            obs_by_dx = {-1: "obsTm", 0: "obsT", 1: "obsTp"}
            for ch in CHANNELS:
                terms = geo[ch]
                if len(terms) == 2:
                    v.tensor_add(t_A[ch][:], sq[terms[0]][:], sq[terms[1]][:])
                    nc.scalar.activation(t_A[ch][:], t_A[ch][:], ACTF.Sqrt,
                                         bias=bias_eps[:], scale=1.0)
                else:
                    nc.scalar.activation(t_A[ch][:], sq[terms[0]][:], ACTF.Sqrt,
                                         bias=bias_eps[:], scale=1.0)
                ody, odx = obsnb[ch]
                nbt = t_in[obs_by_dx[odx]]
                v.tensor_max(t_mx[ch][:], nbt[:, 1 + ody:1 + ody + Dr],
                             t_in["obsT"][:, rows])
                # junk-row slot (free index Dr) to 1e7 first, channels to 0..Dr-1
                v.memset(t_cmapT[ch][:, Dr:KR], INF)
                v.scalar_tensor_tensor(t_cmapT[ch][:, 0:Dr], t_mx[ch][:], OC,
                                       t_A[ch][:], op0=ALU.mult, op1=ALU.add)
                # col -1 lane to 1e7 (used as the junk-col source via sig)
                v.memset(t_cmapT[ch][0:1, :], INF)

            # ---- produce row-major cmap via setup transposes ----
            for ch in CHANNELS:
                scratch = psum_sets[1][0:KR, 0:Sc]
                nc.tensor.matmul(scratch, lhsT=t_cmapT[ch][:], rhs=sigC[:],
                                 is_transpose=True, start=True, stop=True)
                v.tensor_copy(t_cmapR[ch][:, 0:KC], scratch[:, 0:KC])
                v.memset(t_cmapR[ch][:, Dc:KC], INF)   # junk-col lane

            # ---- g0 = clip(INF*(1-start), 0, INF) ----
            v.tensor_scalar(g_rm[0:Dr, 1:1 + Dc], t_start[:], -float(INF), float(INF),
                            op0=ALU.mult, op1=ALU.add)
            v.tensor_scalar_max(g_rm[0:Dr, 1:1 + Dc], g_rm[0:Dr, 1:1 + Dc], 0.0)

            # ---- helpers ----
            def ap3(tile_ap, col_off, dims):
                base = tile_ap
                pap = list(base.ap)
                return bass.AP(base.tensor, base.offset + col_off,
                               [list(pap[0])] + [list(d) for d in dims])

            def preload_A(set_idx):
                # cmap for an odd (g_rm -> s_T) sweep: column-major psum layout
                for dy in (-1, 0, 1):
                    first = True
                    for dx in (-1, 0, 1):
                        if dy == 0 and dx == 0:
                            continue
                        off = (dy + 1) * 512 + (dx + 1) * SS
                        nc.tensor.matmul(
                            psum_sets[set_idx][0:KC, off:off + KR],
                            lhsT=t_cmapR[(dy, dx)][:, 0:KC],
                            rhs=identC[0:KR, 0:KR],
                            is_transpose=True, start=first, stop=False)
                        first = False

            def preload_B(set_idx):
                # cmap for an even (s_T -> g_rm) sweep: row-major psum layout
                for dy in (-1, 0, 1):
                    first = True
                    for dx in (-1, 0, 1):
                        if dy == 0 and dx == 0:
                            continue
                        off = (dy + 1) * 512 + (dx + 1) * SS
                        nc.tensor.matmul(
                            psum_sets[set_idx][0:KR, off:off + Sc],
                            lhsT=t_cmapT[(dy, dx)][:, 0:KR],
                            rhs=sigC[:],
                            is_transpose=True, start=first, stop=False)
                        first = False

            CR = {-1: crmC, 1: crpC}
            CC = {-1: ccmC, 1: ccpC}

            preload_A(0)

            # ---- sweeps ----
            for t in range(1, num_sweeps + 1):
                cur = psum_sets[(t - 1) % 2]
                if t % 2 == 1:
                    # phase A: g_rm -> s_T; windowed over rows
                    lo = max(0, seed_rlo - t)
                    hi = min(Dr - 1, seed_rhi + t)
                    for dx in (-1, 0, 1):
                        for dy in (-1, 0, 1):
                            off = (dy + 1) * 512 + (dx + 1) * SS
                            rhs = identC[0:KR, 0:KR] if dy == 0 else CR[dy][:]
                            nc.tensor.matmul(
                                cur[0:KC, off:off + KR],
                                lhsT=g_rm[:, (1 + dx):(1 + dx) + KC],
                                rhs=rhs,
                                is_transpose=True, start=False, stop=(dx == 1))
                    in_ap = ap3(cur[0:KC, 0:1536], lo, [[1, hi - lo + 1], [512, 3], [SS, 3]])
                    v.tensor_reduce(s_T[:, 1 + lo:1 + hi + 1], in_ap,
                                    axis=AXL.XY, op=ALU.min)
                else:
                    # phase B: s_T -> g_rm; windowed over cols
                    lo = max(0, seed_clo - t)
                    hi = min(Dc - 1, seed_chi + t)
                    for dy in (-1, 0, 1):
                        for dx in (-1, 0, 1):
                            off = (dy + 1) * 512 + (dx + 1) * SS
                            rhs = identC[0:KC, 0:KC] if dx == 0 else CC[dx][:]
                            nc.tensor.matmul(
                                cur[0:KR, off:off + KC],
                                lhsT=s_T[:, (1 + dy):(1 + dy) + KR],
                                rhs=rhs,
                                is_transpose=True, start=False, stop=(dx == 1))
                    in_ap = ap3(cur[0:KR, 0:1536], lo, [[1, hi - lo + 1], [512, 3], [SS, 3]])
                    v.tensor_reduce(g_rm[:, 1 + lo:1 + hi + 1], in_ap,
                                    axis=AXL.XY, op=ALU.min)

                if t < num_sweeps:
                    if t % 2 == 1:
                        preload_B(t % 2)
                    else:
                        preload_A(t % 2)
                    # PE fillers: real-MAC bf16 matmuls bridge the idle gap
                    # between the preloads and the next sweep's shifts; unlike
                    # transpose-mode routing they register as PE activity for
                    # the HAM clock gate (target: hold the 2.4 GHz p-state).
                    for _ in range(2):
                        nc.tensor.matmul(psD[0:128, 0:512],
                                         lhsT=t_warm[0:128, 0:128],
                                         rhs=t_warm[0:128, 0:512],
                                         is_transpose=False,
                                         start=True, stop=True,
                                         skip_group_check=True)

            # ---- final state to row-major if needed, then write out ----
            if num_sweeps % 2 == 1:
                fin = psum_sets[num_sweeps % 2][0:KR, 0:KC]
                nc.tensor.matmul(fin, lhsT=s_T[:, 1:1 + KR],
                                 rhs=identC[0:KC, 0:KC],
                                 is_transpose=True, start=True, stop=True)
                v.tensor_copy(g_rm[0:Dr, 1:1 + Dc], fin[0:Dr, 0:Dc])
            nc.sync.dma_start(out_ap[r0:r0 + Dr, c0:c0 + Dc], g_rm[0:Dr, 1:1 + Dc])

    nc.compile()
    return nc, ["packed"]


def prep_inputs(obstacles, coords, start_map, num_sweeps=NUM_SWEEPS):
    """Host-side slicing/layout prep. Returns (in_map, geometry)."""
    obs = np.asarray(obstacles, np.float32)[0, 0]
    yc = np.asarray(coords, np.float32)[0, 0]
    xc = np.asarray(coords, np.float32)[0, 1]
    s = np.asarray(start_map, np.float32)[0, 0]
    H, W = obs.shape

    ys, xs = np.nonzero(s > 0)
    assert len(ys) >= 1, "empty start_map"
    r0 = max(0, int(ys.min()) - num_sweeps)
    r1 = min(H - 1, int(ys.max()) + num_sweeps)
    c0 = max(0, int(xs.min()) - num_sweeps)
    c1 = min(W - 1, int(xs.max()) + num_sweeps)
    Dr, Dc = r1 - r0 + 1, c1 - c0 + 1
    Sr, Sc = Dr + 2, Dc + 2
    KR, KC = Dr + 1, Dc + 1

    def pad_slice(a):
        ap = np.pad(a, 1, mode='edge')
        return np.ascontiguousarray(ap[r0:r0 + Sr, c0:c0 + Sc], dtype=np.float32)

    obs_p, yc_p, xc_p = pad_slice(obs), pad_slice(yc), pad_slice(xc)

    def tsh(a, dx):
        at = np.ascontiguousarray(a.T)
        if dx == 0:
            return at
        out = np.empty_like(at)
        if dx == -1:
            out[1:] = at[:-1]
            out[0] = at[0]
        else:
            out[:-1] = at[1:]
            out[-1] = at[-1]
        return out

    def cyc(n, d):
        # P[k, j] = 1 iff k == (j + d) mod n
        P = np.zeros((n, n), np.float32)
        P[(np.arange(n) + d) % n, np.arange(n)] = 1.0
        return P

    # sig: out free slot j <- cmapT partition sigma(j);
    # sigma(j) = j+1 for real cols, junk-col slot Dc -> partition 0 (1e7 lane)
    sigma = np.concatenate([np.arange(1, Sc), [0]])
    sigma[Dc] = 0
    sigma[Sc - 1] = Dc + 1
    assert sorted(sigma.tolist()) == list(range(Sc))
    sig = np.zeros((Sc, Sc), np.float32)
    sig[sigma, np.arange(Sc)] = 1.0

    def frame(a, pw):
        out = np.zeros((Sc, pw), np.float32)
        out[0:a.shape[0], 0:a.shape[1]] = a
        return out

    startm = np.zeros((Sc, Dc), np.float32)
    startm[0:Dr, :] = s[r0:r1 + 1, c0:c1 + 1]
    packed = np.concatenate([
        tsh(obs_p, 0), tsh(obs_p, -1), tsh(obs_p, 1),
        tsh(xc_p, 0), tsh(xc_p, -1), tsh(xc_p, 1), tsh(yc_p, 0),
        startm, np.eye(Sc, dtype=np.float32), sig,
        frame(cyc(KR, -1), KR), frame(cyc(KR, 1), KR),
        frame(cyc(KC, -1), KC), frame(cyc(KC, 1), KC),
    ], axis=1)
    in_map = {"packed": np.ascontiguousarray(packed, dtype=np.float32)}

    geom = dict(Dr=Dr, Dc=Dc, r0=r0, c0=c0, H=H, W=W,
                seed_rlo=int(ys.min()) - r0, seed_rhi=int(ys.max()) - r0,
                seed_clo=int(xs.min()) - c0, seed_chi=int(xs.max()) - c0)
    return in_map, geom


def kernel(obstacles, coords, start_map, goal_map):
    in_map, gm = prep_inputs(obstacles, coords, start_map)
    nc, _ = build_program(gm["Dr"], gm["Dc"], gm["seed_rlo"], gm["seed_rhi"],
                          gm["seed_clo"], gm["seed_chi"], gm["r0"], gm["c0"],
                          gm["H"], gm["W"], NUM_SWEEPS)
    in_maps = [in_map for _ in range(N_CORES)]
    res = run_bass_kernel_spmd(nc, in_maps, core_ids=list(range(N_CORES)))
    out = res.results[0]["out"]
    return np.ascontiguousarray(out[None, None]).astype(np.float32)
